# revision 1
# baseline (speedup 1.0000x reference)
"""Trainium2 Bass kernel for nn_DecoderCrossMSA (Swin-style shifted-window
cross-attention).

Strategy: data-parallel over batch (8 batches -> 8 cores). Host prepares, per
core, feature-major window-ordered activations (token axis permuted so every
8x8 shifted window is a contiguous 64-token run; roll folded into the
permutation). Device does:
  - 4 input projections in bf16 (Q scaled by 1/sqrt(32), biases folded where
    possible),
  - windowed attention: S^T = K^T.T @ Q^T per (window, head) on the tensor
    engine, softmax as exp (scalar engine) x static exp-bias table (relative
    position bias + shift masks, multiplicative so masking is exact zeros),
    row-sums via ones-matmul, normalization folded into P,
  - AV matmuls emit feature-major attention output directly,
  - 2 output projections (+ biases folded with V biases) in bf16, fp32 out.
Host inverse-permutes/transposes and reassembles the full outputs.
"""

import os

import numpy as np
import ml_dtypes

EMB = 512
HEADS = 16
WS = 8
B = 8
HW = 64
N = HW * HW
EH = EMB // HEADS          # 32
WN = HW // WS              # 8
SHIFT = WS // 2            # 4
NW = WN * WN               # 64 windows
WT = WS * WS               # 64 tokens per window
NCORES = 8
NBLK = 8                   # token blocks per core (512 tokens each)
BLKT = N // NBLK           # 512
NPAIR = 32                 # window pairs per core
MASK_NEG = -30000.0

_bf16 = ml_dtypes.bfloat16


def _build_perm(shift):
    """perm[t] = token index n for window-ordered position t.

    t = ((i*WN + j) * WT) + (w1*WS + w2); grid row = (WS*i + w1 + shift) mod
    HW, col = (WS*j + w2 + shift) mod HW. Inputs are read through the rolled
    grid (shift=SHIFT); outputs are written back WITHOUT inverting the roll
    (shift=0) — the reference's _unwindow does not undo the roll.
    """
    i, j, w1, w2 = np.meshgrid(
        np.arange(WN), np.arange(WN), np.arange(WS), np.arange(WS), indexing="ij"
    )
    r = (WS * i + w1 + shift) % HW
    c = (WS * j + w2 + shift) % HW
    return (r * HW + c).reshape(-1)


_PERM = _build_perm(SHIFT)
_OPERM = _build_perm(0)

# Reference splits EMB as (e H): head h lives on strided channels e*HEADS+h.
# Permute projection out-channels so head h is the contiguous block h*EH..:
# new channel h*EH+e = old channel e*HEADS+h.
_RHO = np.array([e * HEADS + h for h in range(HEADS) for e in range(EH)])


def _pair_tables(pos_emb):
    """4 pair-type tables [128, 16*64] bf16 of exp(T)^T, head-replicated.

    T[q, k] = pos_bias[q, k] (+ row mask if window-row i == WN-1)
                         (+ col mask if window-col j == WN-1).
    Table rows = k (2 windows stacked: first window of pair rows 0:64, second
    rows 64:128), free = (16 heads replicated, 64 q).
    pair p = windows (2p, 2p+1): second window is col-masked iff p % 4 == 3;
    both windows row-masked iff p // 4 == WN - 1.
    """
    idx = np.array([[x, y] for x in range(WS) for y in range(WS)])
    rel = idx[None, :, :] - idx[:, None, :] + WS - 1
    bias = pos_emb[rel[:, :, 0], rel[:, :, 1]].astype(np.float64)

    m = np.zeros((WT, WT), dtype=np.float64)
    s = WS * (WS // 2)
    m[-s:, :-s] = MASK_NEG
    m[:-s, -s:] = MASK_NEG
    r = WT // WS
    col = m.reshape(r, WS, r, WS).transpose(1, 0, 3, 2).reshape(WT, WT)

    t0 = bias
    t1 = bias + m          # row-masked  (i == 7)
    t2 = bias + col        # col-masked  (j == 7)
    t3 = bias + m + col    # corner

    def pair_tab(ta, tb):
        # exp, transpose to [k, q], stack windows, replicate over heads
        ea = np.exp(ta).T    # [k, q]
        eb = np.exp(tb).T
        stk = np.concatenate([ea, eb], axis=0)           # [128, 64]
        rep = np.tile(stk, (1, HEADS))                    # [128, 16*64]
        return rep.astype(_bf16)

    # pair types: (normal,normal), (normal,colmask), (rowmask,rowmask),
    # (rowmask,corner)
    return np.stack([
        pair_tab(t0, t0),
        pair_tab(t0, t2),
        pair_tab(t1, t1),
        pair_tab(t1, t3),
    ])


def _pair_type(p):
    row = (p // 4) == WN - 1      # window-row i == 7
    colm = (p % 4) == 3           # second window j == 7
    return (2 if row else 0) + (1 if colm else 0)


def _build_bass(debug=False, stage=99, reps=1):
    import concourse.mybir as mybir
    from concourse import bacc
    from concourse.tile import TileContext

    fp32 = mybir.dt.float32
    bf16 = mybir.dt.bfloat16
    AF = mybir.ActivationFunctionType
    ALU = mybir.AluOpType

    nc = bacc.Bacc()

    # ---- DRAM parameters (per-core) ----
    d_in = {}
    for name in ("cw", "sw", "scw", "shw"):
        d_in[name] = nc.declare_dram_parameter(name, [EMB, N], bf16, isOutput=False)
    for name in ("w1t", "w2t", "wsct", "wsht", "wsot", "wshot"):
        d_in[name] = nc.declare_dram_parameter(name, [EMB, EMB], bf16, isOutput=False)
    for name in ("b1r", "b2r", "bsor", "bshor"):
        d_in[name] = nc.declare_dram_parameter(name, [128, 4], fp32, isOutput=False)
    d_in["ptab"] = nc.declare_dram_parameter(
        "ptab", [4, 128, HEADS * WT], bf16, isOutput=False
    )
    d_in["onesc"] = nc.declare_dram_parameter("onesc", [128, WT], bf16, isOutput=False)
    yso = nc.declare_dram_parameter("yso", [EMB, N], fp32, isOutput=True)
    ysho = nc.declare_dram_parameter("ysho", [EMB, N], fp32, isOutput=True)
    dbg = {}
    if debug:
        for name, shape in (
            ("dbg_ct", [EMB, BLKT]), ("dbg_st", [128, HEADS * WT]),
            ("dbg_pa", [128, HEADS * WT]), ("dbg_pn", [128, HEADS * WT]),
            ("dbg_av", [128, 1024]), ("dbg_v", [128, EMB]),
        ):
            dbg[name] = nc.declare_dram_parameter(name, shape, fp32, isOutput=True)

    with TileContext(nc) as tc:
        with (
            tc.tile_pool(name="const", bufs=1) as cpool,
            tc.tile_pool(name="xg", bufs=2) as xgpool,
            tc.tile_pool(name="cs", bufs=2) as cspool,
            tc.tile_pool(name="v", bufs=5) as vpool,
            tc.tile_pool(name="p", bufs=4) as ppool,
            tc.tile_pool(name="o", bufs=3) as opool,
            tc.tile_pool(name="y", bufs=4) as ypool,
            tc.tile_pool(name="stps", bufs=1, space="PSUM") as stps,
            tc.tile_pool(name="bigps", bufs=2, space="PSUM") as bigps,
        ):
            # ---- constants into SBUF ----
            wts = {}
            for name in ("w1t", "w2t", "wsct", "wsht", "wsot", "wshot"):
                wts[name] = []
                for k in range(4):
                    t = cpool.tile([128, EMB], bf16, tag=f"{name}_{k}")
                    nc.sync.dma_start(t[:], d_in[name][k * 128:(k + 1) * 128, :])
                    wts[name].append(t)
            bias_t = {}
            for name in ("b1r", "b2r", "bsor", "bshor"):
                t = cpool.tile([128, 4], fp32, tag=name)
                nc.sync.dma_start(t[:], d_in[name][:])
                bias_t[name] = t
            ptab_t = []
            for i in range(4):
                t = cpool.tile([128, HEADS * WT], bf16, tag=f"ptab{i}")
                nc.sync.dma_start(t[:], d_in["ptab"][i])
                ptab_t.append(t)
            ones_t = cpool.tile([128, WT], bf16, tag="onesc")
            nc.sync.dma_start(ones_t[:], d_in["onesc"][:])

            for blk0 in range(NBLK * reps):
                blk = blk0 % NBLK
                c0 = blk * BLKT
                # ---- stage inputs [e_in chunk, 512 tokens] ----
                xg = {}
                for tname in ("cw", "sw", "scw", "shw"):
                    xg[tname] = []
                    for k in range(4):
                        t = xgpool.tile([128, BLKT], bf16, tag=f"xg_{tname}_{k}")
                        nc.sync.dma_start(
                            t[:], d_in[tname][k * 128:(k + 1) * 128, c0:c0 + BLKT]
                        )
                        xg[tname].append(t)

                # ---- Q/K projections (feature-major) ----
                cs = {}
                for tname, wname, bname in (
                    ("cw", "w1t", "b1r"), ("sw", "w2t", "b2r")
                ):
                    cs[tname] = []
                    for m in range(4):
                        ps = bigps.tile([128, BLKT], fp32, tag="big", name="pspj")
                        for k in range(4):
                            nc.tensor.matmul(
                                ps[:],
                                lhsT=wts[wname][k][:, m * 128:(m + 1) * 128],
                                rhs=xg[tname][k][:],
                                start=(k == 0),
                                stop=(k == 3),
                            )
                        out = cspool.tile([128, BLKT], bf16, tag=f"cs_{tname}_{m}")
                        nc.scalar.activation(
                            out[:], ps[:], AF.Identity,
                            bias=bias_t[bname][:, m:m + 1],
                        )
                        cs[tname].append(out)
                cT, sT = cs["cw"], cs["sw"]
                if debug and blk == 0:
                    for m in range(4):
                        nc.gpsimd.dma_start(
                            dbg["dbg_ct"][m * 128:(m + 1) * 128, :], cT[m][:]
                        )

                # ---- V projections (token-major), per pair ----
                vsc_l, vsh_l = [], []
                for p in range(4):
                    t0 = p * 128
                    for tname, wname, dst in (
                        ("scw", "wsct", vsc_l), ("shw", "wsht", vsh_l)
                    ):
                        ps = bigps.tile([128, EMB], fp32, tag="big", name="psv")
                        for k in range(4):
                            nc.tensor.matmul(
                                ps[:],
                                lhsT=xg[tname][k][:, t0:t0 + 128],
                                rhs=wts[wname][k][:],
                                start=(k == 0),
                                stop=(k == 3),
                            )
                        out = vpool.tile([128, EMB], bf16, tag=f"v_{tname}")
                        nc.vector.tensor_copy(out[:], ps[:])
                        dst.append(out)

                if stage <= 1:
                    continue
                # ---- attention per pair ----
                osc = opool.tile([128, 4 * BLKT], bf16, tag="osc")
                osh = opool.tile([128, 4 * BLKT], bf16, tag="osh")
                for p in range(4):
                    pg = blk * 4 + p
                    ptype = _pair_type(pg)
                    t0 = p * 128
                    # S^T psum: 4 banks; head h lands in bank h%4 == its PE
                    # row-group, so concurrent row-tiled matmuls never share a
                    # (bank, partition) pair (HW write-collision otherwise).
                    st = stps.tile([128, 4 * 512], fp32, tag="st")
                    for h in range(HEADS):
                        m, r = h // 4, (h % 4) * 32
                        s0 = (h % 4) * 512 + (h // 4) * WT
                        for wi in range(2):
                            o0 = t0 + wi * WT
                            nc.tensor.matmul(
                                st[wi * WT:(wi + 1) * WT, s0:s0 + WT],
                                lhsT=sT[m][r:r + 32, o0:o0 + WT],
                                rhs=cT[m][r:r + 32, o0:o0 + WT],
                                start=True, stop=True,
                                tile_position=(r, wi * WT),
                            )
                    # compact [128, 4, 4, 64] view of the used st slots
                    st_v = st[:].rearrange(
                        "p (b s q) -> p b s q", b=4, s=8, q=WT
                    )[:, :, 0:4, :]
                    if debug and blk == 0 and p == 0:
                        t = ypool.tile([128, HEADS * WT], fp32, tag="dbg")
                        tv = t[:].rearrange("p (b s q) -> p b s q", b=4, s=4, q=WT)
                        nc.scalar.activation(tv, st_v, AF.Copy)
                        nc.gpsimd.dma_start(dbg["dbg_st"][:], t[:])
                    pe = ppool.tile([128, HEADS * WT], bf16, tag="pe")
                    pe_v = pe[:].rearrange("p (b s q) -> p b s q", b=4, s=4, q=WT)
                    nc.scalar.activation(pe_v, st_v, AF.Exp)
                    pa = ppool.tile([128, HEADS * WT], bf16, tag="pa")
                    nc.vector.tensor_tensor(
                        pa[:], pe[:], ptab_t[ptype][:], ALU.mult
                    )
                    if stage <= 2:
                        continue
                    dd = bigps.tile([128, HEADS * WT], fp32, tag="big", name="dd")
                    for wi in range(2):
                        sl = slice(wi * WT, (wi + 1) * WT)
                        for half in range(2):
                            fs = slice(half * 512, (half + 1) * 512)
                            nc.tensor.matmul(
                                dd[sl, fs],
                                lhsT=ones_t[sl, :],
                                rhs=pa[sl, fs],
                                start=True, stop=True,
                                tile_position=(wi * WT, wi * WT),
                            )
                    rd = ppool.tile([128, HEADS * WT], fp32, tag="rd")
                    nc.vector.reciprocal(rd[:], dd[:])
                    pn = ppool.tile([128, HEADS * WT], bf16, tag="pn")
                    nc.vector.tensor_tensor(pn[:], pa[:], rd[:], ALU.mult)
                    if debug and blk == 0 and p == 0:
                        nc.gpsimd.dma_start(dbg["dbg_pa"][:], pa[:])
                        nc.gpsimd.dma_start(dbg["dbg_pn"][:], pn[:])
                        nc.gpsimd.dma_start(dbg["dbg_v"][:], vsc_l[0][:])
                    if stage <= 3:
                        continue

                    # AV psum: bank = window half == PE row-group of the MM.
                    # free = wi*512 + m*128 + q
                    av_sc = bigps.tile([128, 1024], fp32, tag="big", name="avsc")
                    av_sh = bigps.tile([128, 1024], fp32, tag="big", name="avsh")
                    for h in range(HEADS):
                        m, r = h // 4, (h % 4) * 32
                        ps0 = ((h % 4) * 4 + h // 4) * WT   # pn slot for head h
                        for wi in range(2):
                            sl = slice(wi * WT, (wi + 1) * WT)
                            f0 = wi * 512 + m * WT
                            for vt, av in ((vsc_l[p], av_sc), (vsh_l[p], av_sh)):
                                nc.tensor.matmul(
                                    av[r:r + 32, f0:f0 + WT],
                                    lhsT=vt[sl, h * 32:(h + 1) * 32],
                                    rhs=pn[sl, ps0:ps0 + WT],
                                    start=True, stop=True,
                                    tile_position=(wi * WT, r),
                                )
                    if debug and blk == 0 and p == 0:
                        t = ypool.tile([128, 1024], fp32, tag="dbg2")
                        nc.scalar.activation(t[:], av_sc[:], AF.Copy)
                        nc.gpsimd.dma_start(dbg["dbg_av"][:], t[:])
                    # scatter [128, (2 wi, 4 m, 64 q)] into O tiles
                    for o_t, av, eng in ((osc, av_sc, nc.scalar),
                                         (osh, av_sh, nc.vector)):
                        src = av[:].rearrange("p (w m q) -> p m w q", w=2, m=8,
                                              q=WT)[:, 0:4]
                        dstv = o_t[:].rearrange("p (m t) -> p m t", m=4)
                        dst = dstv[:, :, t0:t0 + 128].rearrange(
                            "p m (w q) -> p m w q", w=2
                        )
                        if eng is nc.scalar:
                            nc.scalar.activation(dst, src, AF.Copy)
                        else:
                            nc.vector.tensor_copy(dst, src)

                if stage <= 4:
                    continue
                # ---- output projections ----
                for o_t, wname, bname, y_h in (
                    (osc, "wsot", "bsor", yso), (osh, "wshot", "bshor", ysho)
                ):
                    for mo in range(4):
                        ps = bigps.tile([128, BLKT], fp32, tag="big", name="pso")
                        for k in range(4):
                            nc.tensor.matmul(
                                ps[:],
                                lhsT=wts[wname][k][:, mo * 128:(mo + 1) * 128],
                                rhs=o_t[:, k * BLKT:(k + 1) * BLKT],
                                start=(k == 0),
                                stop=(k == 3),
                            )
                        y_sb = ypool.tile([128, BLKT], fp32, tag="y")
                        nc.scalar.activation(
                            y_sb[:], ps[:], AF.Identity,
                            bias=bias_t[bname][:, mo:mo + 1],
                        )
                        nc.sync.dma_start(
                            y_h[mo * 128:(mo + 1) * 128, c0:c0 + BLKT], y_sb[:]
                        )
    nc.compile()
    return nc


_NC_CACHE = {}
LAST_RESULT = None


def make_in_maps(content, style, scale, shift, W1, b1, W2, b2, Wsc, bsc,
                 Wsh, bsh, Wso, bso, Wsho, bsho, pos_emb):
    inv = 1.0 / np.sqrt(EMB / HEADS)
    f32 = np.float32

    # head-contiguous channel permutation on projection out-channels (_RHO);
    # inverted on the output-projection in-channels.
    w1t = (np.asarray(W1, f32)[_RHO].T * inv).astype(_bf16)  # [e_in, e_out], scaled
    w2t = np.asarray(W2, f32)[_RHO].T.astype(_bf16)
    wsct = np.asarray(Wsc, f32)[_RHO].T.astype(_bf16)
    wsht = np.asarray(Wsh, f32)[_RHO].T.astype(_bf16)
    # _unwindow emits channels H-major (h*EH+e) == device O-row order, so the
    # output projections are NOT channel-permuted.
    wsot = np.asarray(Wso, f32).T.astype(_bf16)
    wshot = np.asarray(Wsho, f32).T.astype(_bf16)
    b1r = (np.asarray(b1, f32)[_RHO] * inv).reshape(4, 128).T.copy()
    b2r = np.asarray(b2, f32)[_RHO].reshape(4, 128).T.copy()
    # V biases folded into output-projection biases; V channels reach the
    # output projection in unwindow (H-major) order, hence bsc[_RHO].
    bso2 = np.asarray(Wso, f32) @ np.asarray(bsc, f32)[_RHO] + np.asarray(bso, f32)
    bsho2 = (np.asarray(Wsho, f32) @ np.asarray(bsh, f32)[_RHO]
             + np.asarray(bsho, f32))
    bsor = bso2.reshape(4, 128).T.copy()
    bshor = bsho2.reshape(4, 128).T.copy()
    ptab = _pair_tables(np.asarray(pos_emb, f32))
    onesc = np.ones((128, WT), dtype=_bf16)

    common = dict(
        w1t=w1t, w2t=w2t, wsct=wsct, wsht=wsht, wsot=wsot, wshot=wshot,
        b1r=b1r, b2r=b2r, bsor=bsor, bshor=bshor, ptab=ptab, onesc=onesc,
    )
    in_maps = []
    for b in range(NCORES):
        m = dict(common)
        for name, full in (("cw", content), ("sw", style),
                           ("scw", scale), ("shw", shift)):
            x = np.asarray(full[b], f32)[_PERM]           # [N, EMB] window order
            m[name] = np.ascontiguousarray(x.T).astype(_bf16)
        in_maps.append(m)
    return in_maps


def kernel(**inputs):
    global LAST_RESULT
    from concourse.bass_utils import run_bass_kernel_spmd

    in_maps = make_in_maps(**inputs)

    if "nc" not in _NC_CACHE:
        _NC_CACHE["nc"] = _build_bass()
    res = run_bass_kernel_spmd(_NC_CACHE["nc"], in_maps, list(range(NCORES)))
    LAST_RESULT = res

    out_sc = np.empty((B, N, EMB), np.float32)
    out_sh = np.empty((B, N, EMB), np.float32)
    for b in range(NCORES):
        out_sc[b][_OPERM] = res.results[b]["yso"].T
        out_sh[b][_OPERM] = res.results[b]["ysho"].T
    return out_sc, out_sh

